# revision 1
# baseline (speedup 1.0000x reference)
"""GPT2 attention (B=2,S=2048,D=1024,H=16,hd=64, no causal mask) on 8 trn2 cores.

Sharding: core c handles batch b=c//4 and head-group g=c%4 (4 heads).
w_attn columns are split per head group (Q scaled by 1/sqrt(hd) on host);
w_proj rows split per head group; host sums the 4 partial c_proj outputs
per batch (the "all-reduce").

Per-core dataflow (matmuls in float32r, 1 cyc/row at N>=512; every tile a
matmul consumes is written as float32r by its producer so walrus' rounding
check passes):
  hid [2048,1024] --PE transpose--> hidT [1024,2048]
  qkvT[768,2048] = w_slice.T @ hidT   (feature-major Q^T,K^T,V^T, 2 heads/tile)
  V^T --PE transpose--> vaug [k,65] tiles (col 64 = ones for denominator)
  per (head, 512-wide q chunk):
    S^T[k,q] tiles = K^T_tile.T @ Q^T  -> DVE copy to SBUF block [128, 4096]
    one ACT exp per block (amortizes ACT fixed cost; no max-subtraction:
    scores are O(1) so exp is numerically safe)
    O_u^T[65,512] = sum_k vaug.T @ E   (row 64 = softmax denominator)
    obar_h = O_u^T[0:64] * broadcast(1/denom)  (ones-matmul broadcast + DVE mul)
  out[q,1024] = sum_h obar_h.T @ wp_h  (K=64 accumulation, 4 heads)
"""

import sys

import numpy as np

if "/opt/trn_rl_repo" not in sys.path:
    sys.path.insert(0, "/opt/trn_rl_repo")

S = 2048
D = 1024
P = 128
NH = 4  # heads per core
HD = 64
N_CORES = 8

_CACHE = {}


def _build_program():
    import concourse.mybir as mybir
    from concourse import bacc
    from concourse.masks import make_identity
    from concourse.tile import TileContext

    f32r = mybir.dt.float32r
    f32 = mybir.dt.float32
    AF = mybir.ActivationFunctionType
    ALU = mybir.AluOpType

    nc = bacc.Bacc(None, target_bir_lowering=False, debug=False)
    hid = nc.declare_dram_parameter("hid", [S, D], f32r, isOutput=False)
    wqkv = nc.declare_dram_parameter("wqkv", [D, 3 * NH * HD], f32r, isOutput=False)
    wp = nc.declare_dram_parameter("wp", [NH * HD, D], f32r, isOutput=False)
    out = nc.declare_dram_parameter("out", [S, D], f32, isOutput=True)

    with TileContext(nc) as tc:
        with tc.tile_pool(name="const", bufs=1) as constp:
            ident_f = constp.tile([P, P], f32)
            make_identity(nc, ident_f)
            ident = constp.tile([P, P], f32r)
            nc.vector.tensor_copy(ident[:], ident_f[:])
            ones_f = constp.tile([P, HD], f32)
            nc.gpsimd.memset(ones_f[:], 1.0)
            ones_t = constp.tile([P, HD], f32r)
            nc.vector.tensor_copy(ones_t[:], ones_f[:])

            qkvT = [constp.tile([P, S], f32r, name=f"qkvT{i}") for i in range(6)]
            vaug = constp.tile([P, NH * 16 * 65], f32r)

            # ---------------- Stage A: hidT + QKV ----------------
            with tc.tile_pool(name="hidT_pool", bufs=1) as hidTp, \
                 tc.tile_pool(name="stageA", bufs=3) as sA, \
                 tc.tile_pool(name="w_pool", bufs=1) as wpool, \
                 tc.tile_pool(name="tpsum", bufs=3, space="PSUM") as tpsum, \
                 tc.tile_pool(name="qpsum", bufs=3, space="PSUM") as qpsum:
                hidT = [hidTp.tile([P, S], f32r, name=f"hidT{i}") for i in range(8)]
                w_sb = [wpool.tile([P, 768], f32r, name=f"w{i}") for i in range(8)]
                for i in range(8):
                    nc.sync.dma_start(out=w_sb[i][:], in_=wqkv[i * P : (i + 1) * P, :])
                for st in range(16):
                    ht = sA.tile([P, D], f32r, tag="hidload")
                    nc.sync.dma_start(out=ht[:], in_=hid[st * P : (st + 1) * P, :])
                    for dt_ in range(8):
                        tp = tpsum.tile([P, P], f32r, tag="tp")
                        nc.tensor.transpose(
                            tp[:], ht[:, dt_ * P : (dt_ + 1) * P], ident[:]
                        )
                        nc.vector.tensor_copy(
                            hidT[dt_][:, st * P : (st + 1) * P], tp[:]
                        )
                for ct in range(6):
                    for qc in range(4):
                        ps = qpsum.tile([P, 512], f32, tag="qkvps")
                        for dt_ in range(8):
                            nc.tensor.matmul(
                                ps[:],
                                lhsT=w_sb[dt_][:, ct * P : (ct + 1) * P],
                                rhs=hidT[dt_][:, qc * 512 : (qc + 1) * 512],
                                start=(dt_ == 0),
                                stop=(dt_ == 7),
                            )
                        nc.vector.tensor_copy(
                            qkvT[ct][:, qc * 512 : (qc + 1) * 512], ps[:]
                        )
                # V seq-major (transpose V^T) into vaug; col 64 of each 65 = ones
                for h in range(NH):
                    par = HD * (h % 2)
                    vsrc = qkvT[4 + h // 2]
                    for kt in range(16):
                        vp = tpsum.tile([P, P], f32r, tag="tp")
                        nc.tensor.transpose(
                            vp[:, :HD],
                            vsrc[par : par + HD, kt * P : (kt + 1) * P],
                            ident[par : par + HD, par : par + HD],
                        )
                        base = (h * 16 + kt) * 65
                        nc.vector.tensor_copy(vaug[:, base : base + HD], vp[:, :HD])
                        nc.vector.tensor_copy(
                            vaug[:, base + HD : base + 65], ones_f[:, 0:1]
                        )

            # ---------------- Stages B+C ----------------
            with tc.tile_pool(name="persistBC", bufs=1) as perBC:
                obar = [perBC.tile([HD, S], f32r, name=f"obar{i}") for i in range(NH)]
                wp_sb = [perBC.tile([HD, D], f32r, name=f"wp{i}") for i in range(NH)]
                for h in range(NH):
                    nc.sync.dma_start(
                        out=wp_sb[h][:], in_=wp[h * HD : (h + 1) * HD, :]
                    )

                with tc.tile_pool(name="sblk", bufs=3) as sblk, \
                     tc.tile_pool(name="npool", bufs=3) as npool, \
                     tc.tile_pool(name="spsum", bufs=2, space="PSUM") as spsum, \
                     tc.tile_pool(name="opsum", bufs=1, space="PSUM") as opsum, \
                     tc.tile_pool(name="rpsum", bufs=1, space="PSUM") as rpsum:
                    for h in range(NH):
                        par = HD * (h % 2)
                        qT = qkvT[0 + h // 2]
                        kT = qkvT[2 + h // 2]
                        for qc in range(2):
                            q0 = qc * 1024
                            op = opsum.tile([65, 1024], f32, tag="op")
                            for kt in range(16):
                                sp = spsum.tile([P, 1024], f32, tag="sp")
                                for u in range(2):
                                    nc.tensor.matmul(
                                        sp[:, u * 512 : (u + 1) * 512],
                                        lhsT=kT[par : par + HD, kt * P : (kt + 1) * P],
                                        rhs=qT[par : par + HD, q0 + u * 512 : q0 + (u + 1) * 512],
                                        start=True,
                                        stop=True,
                                    )
                                eb = sblk.tile([P, 1024], f32r, tag="sb")
                                nc.scalar.activation(eb[:], sp[:], AF.Exp)
                                base = (h * 16 + kt) * 65
                                for u in range(2):
                                    nc.tensor.matmul(
                                        op[:, u * 512 : (u + 1) * 512],
                                        lhsT=vaug[:, base : base + 65],
                                        rhs=eb[:, u * 512 : (u + 1) * 512],
                                        start=(kt == 0),
                                        stop=(kt == 15),
                                    )
                            rec = npool.tile([P, 1024], f32r, tag="rec")
                            with nc.allow_low_precision(
                                reason="f32r recip of softmax denom"
                            ):
                                nc.vector.reciprocal(rec[64:65, :], op[64:65, :])
                            rb = rpsum.tile([HD, 1024], f32, tag="rb")
                            for u in range(2):
                                nc.tensor.matmul(
                                    rb[:, u * 512 : (u + 1) * 512],
                                    lhsT=ones_t[64:65, :],
                                    rhs=rec[64:65, u * 512 : (u + 1) * 512],
                                    start=True, stop=True,
                                )
                            ou_sb = npool.tile([HD, 1024], f32r, tag="ou")
                            nc.vector.tensor_copy(ou_sb[:], op[0:HD, :])
                            rb_sb = npool.tile([HD, 1024], f32r, tag="rbs")
                            nc.vector.tensor_copy(rb_sb[:], rb[:])
                            with nc.allow_low_precision(
                                reason="softmax normalize in f32r"
                            ):
                                nc.vector.tensor_tensor(
                                    out=obar[h][:, q0 : q0 + 1024],
                                    in0=ou_sb[:],
                                    in1=rb_sb[:],
                                    op=ALU.mult,
                                )

                # ---------------- Stage C: projection ----------------
                with tc.tile_pool(name="outp", bufs=4) as outp, \
                     tc.tile_pool(name="ppsum", bufs=4, space="PSUM") as ppsum:
                    for qt in range(16):
                        ot = outp.tile([P, D], f32, tag="ot")
                        for ec in range(2):
                            pp = ppsum.tile([P, 512], f32, tag="pp")
                            for h in range(NH):
                                nc.tensor.matmul(
                                    pp[:],
                                    lhsT=obar[h][:, qt * P : (qt + 1) * P],
                                    rhs=wp_sb[h][:, ec * 512 : (ec + 1) * 512],
                                    start=(h == 0),
                                    stop=(h == NH - 1),
                                )
                            nc.vector.tensor_copy(
                                ot[:, ec * 512 : (ec + 1) * 512], pp[:]
                            )
                        nc.sync.dma_start(
                            out=out[qt * P : (qt + 1) * P, :], in_=ot[:]
                        )

    nc.compile()
    return nc


def _get_nc():
    if "nc" not in _CACHE:
        _CACHE["nc"] = _build_program()
    return _CACHE["nc"]


def _shard_inputs(hidden_states, w_attn, w_proj):
    scale = 1.0 / np.sqrt(np.float32(HD))
    in_maps = []
    for c in range(N_CORES):
        b, g = divmod(c, 4)
        cs = slice(g * NH * HD, (g + 1) * NH * HD)
        wq = w_attn[:, 0:D][:, cs] * scale
        wk = w_attn[:, D : 2 * D][:, cs]
        wv = w_attn[:, 2 * D : 3 * D][:, cs]
        in_maps.append(
            {
                "hid": np.ascontiguousarray(hidden_states[b], dtype=np.float32),
                "wqkv": np.ascontiguousarray(
                    np.concatenate([wq, wk, wv], axis=1), dtype=np.float32
                ),
                "wp": np.ascontiguousarray(w_proj[cs, :], dtype=np.float32),
            }
        )
    return in_maps


def run(hidden_states, w_attn, w_proj, trace=False):
    from concourse.bass_utils import run_bass_kernel_spmd

    nc = _get_nc()
    in_maps = _shard_inputs(hidden_states, w_attn, w_proj)
    res = run_bass_kernel_spmd(nc, in_maps, list(range(N_CORES)), trace=trace)
    parts = [res.results[c]["out"] for c in range(N_CORES)]
    out = np.stack(
        [
            parts[0] + parts[1] + parts[2] + parts[3],
            parts[4] + parts[5] + parts[6] + parts[7],
        ]
    ).astype(np.float32)
    return out, res


def kernel(hidden_states, w_attn, w_proj):
    out, _ = run(
        np.asarray(hidden_states), np.asarray(w_attn), np.asarray(w_proj)
    )
    return out



# revision 10
# speedup vs baseline: 1.5442x; 1.5442x over previous
"""GPT2 attention (B=2,S=2048,D=1024,H=16,hd=64, no causal mask) on 8 trn2 cores.

Sharding: core c handles batch b=c//4 and head-group g=c%4 (4 heads).
w_attn columns split per head group (Q pre-scaled by 1/sqrt(hd) on host);
w_proj rows split per head group; host sums the 4 partial c_proj outputs
per batch.

All matmul operands are bf16 (rel-err budget is 2e-2 rms; bf16 lands ~1e-3).
bf16 enables fast weight load, so per-matmul LDWEIGHTS hides behind the
matmul stream, and halves DVE/SBUF/DMA traffic vs f32.

Host-side prep: hid is shipped pre-transposed (hidT [D,S]) so the kernel
needs no PE transposes at all; the c_proj output is produced feature-major
(outT [D,S]) and transposed back on host.

Per-core dataflow:
  A) V seq-major: vps[st,:256] = hidT_tiles.T @ wv  -> vaug [128k, 65] tiles
     (col 64 pre-set to ones -> PV matmul row 64 = softmax denominator)
     Q,K feature-major: qkT[ct][128,2048] = w_slice.T @ hidT (2 heads/tile)
  B) flash loop, per (q-chunk 512, head-pair): 16 k-tiles:
     scores: two row-tiled (K=64) matmuls (head pair runs concurrently in
     the PE array) -> sp [128,1024] PSUM; one ACT exp -> eb bf16;
     PV: op[65, 512+512] += vaug.T @ eb  (row 64 = denominator)
     normalize: denominators batched -> one DVE reciprocal per q-chunk,
     ones-matmul broadcast, DVE multiply -> obar bf16
  C) c_proj feature-major: outT[et,qs] += wp_h.T @ obar_h, drain bf16,
     DMA out. Overlaps stage B of later q-chunks.
"""

import sys

import numpy as np

if "/opt/trn_rl_repo" not in sys.path:
    sys.path.insert(0, "/opt/trn_rl_repo")

S = 2048
D = 1024
P = 128
NH = 4  # heads per core
HD = 64
N_CORES = 8
QC = 512  # q-chunk width
NQC = S // QC  # 4
NKT = S // P  # 16 k-tiles

_CACHE = {}


def _build_program():
    import concourse.mybir as mybir
    from concourse import bacc
    from concourse.tile import TileContext

    bf16 = mybir.dt.bfloat16
    f32 = mybir.dt.float32
    AF = mybir.ActivationFunctionType
    ALU = mybir.AluOpType

    nc = bacc.Bacc(None, target_bir_lowering=False, debug=False)
    hidT = nc.declare_dram_parameter("hidT", [D, S], bf16, isOutput=False)
    wqkv = nc.declare_dram_parameter("wqkv", [D, 3 * NH * HD], bf16, isOutput=False)
    wp = nc.declare_dram_parameter("wp", [NH * HD, D], bf16, isOutput=False)
    outT = nc.declare_dram_parameter("outT", [D, S], bf16, isOutput=True)

    with TileContext(nc) as tc:
        with tc.tile_pool(name="const", bufs=1) as constp:
            ones_bc = constp.tile([P, HD], bf16)
            nc.gpsimd.memset(ones_bc[:], 1.0)
            # vaug: per (h, kt) a [128, 65] block: cols 0..63 = V rows,
            # col 64 = ones (PV denominator row). Pre-set everything to 1;
            # V copies overwrite cols 0..63.
            vaug = constp.tile([P, NH * NKT * 65], bf16)
            nc.gpsimd.memset(vaug[:], 1.0)

            # persistent SBUF across stages
            qkT = [constp.tile([P, S], bf16, name=f"qkT{i}") for i in range(4)]
            obar = [constp.tile([HD, S], bf16, name=f"obar{i}") for i in range(NH)]
            wp_sb = [constp.tile([HD, D], bf16, name=f"wp{i}") for i in range(NH)]
            # denominators: head h lives on partition 32h so the K=1
            # broadcast matmul's tile_position lands 32-aligned
            dden = constp.tile([97, S], f32)
            nc.gpsimd.memset(dden[:], 1.0)
            drec = constp.tile([97, S], bf16)
            for h in range(NH):
                nc.sync.dma_start(out=wp_sb[h][:], in_=wp[h * HD : (h + 1) * HD, :])

            # ---------------- Stage A ----------------
            with tc.tile_pool(name="hid_pool", bufs=1) as hidp, \
                 tc.tile_pool(name="w_pool", bufs=1) as wpool, \
                 tc.tile_pool(name="vpsum", bufs=3, space="PSUM") as vpsum, \
                 tc.tile_pool(name="qpsum", bufs=1, space="PSUM") as qpsum:
                hid_sb = [hidp.tile([P, S], bf16, name=f"hidT{i}") for i in range(8)]
                w_sb = [wpool.tile([P, 3 * NH * HD], bf16, name=f"w{i}") for i in range(8)]
                for i in range(8):
                    nc.sync.dma_start(out=hid_sb[i][:], in_=hidT[i * P : (i + 1) * P, :])
                    nc.sync.dma_start(out=w_sb[i][:], in_=wqkv[i * P : (i + 1) * P, :])

                # V seq-major: tiles [128 s, 256] accumulated over d-chunks
                for st in range(NKT):
                    vps = vpsum.tile([P, NH * HD], f32, tag="vps")
                    for dt_ in range(8):
                        nc.tensor.matmul(
                            vps[:],
                            lhsT=hid_sb[dt_][:, st * P : (st + 1) * P],
                            rhs=w_sb[dt_][:, 2 * NH * HD : 3 * NH * HD],
                            start=(dt_ == 0),
                            stop=(dt_ == 7),
                        )
                    for h in range(NH):
                        base = (h * NKT + st) * 65
                        nc.vector.tensor_copy(
                            vaug[:, base : base + HD], vps[:, h * HD : (h + 1) * HD]
                        )

                # Q,K feature-major; ct: 0=Q(h0,h1) 1=Q(h2,h3) 2=K(h0,h1) 3=K(h2,h3)
                for ct in (0, 2, 1, 3):
                    ps = [
                        qpsum.tile([P, QC], f32, tag=f"qk{q}", name=f"qkps{q}")
                        for q in range(4)
                    ]
                    for dt_ in range(8):
                        for q in range(4):
                            nc.tensor.matmul(
                                ps[q][:],
                                lhsT=w_sb[dt_][:, ct * P : (ct + 1) * P],
                                rhs=hid_sb[dt_][:, q * QC : (q + 1) * QC],
                                start=(dt_ == 0),
                                stop=(dt_ == 7),
                            )
                    for q in range(4):
                        nc.scalar.copy(qkT[ct][:, q * QC : (q + 1) * QC], ps[q][:])

            # ---------------- Stages B+C ----------------
            ou_tiles = {}
            with tc.tile_pool(name="ebp", bufs=3) as ebp, \
                 tc.tile_pool(name="oup", bufs=6) as oup, \
                 tc.tile_pool(name="otp", bufs=4) as otp, \
                 tc.tile_pool(name="scratch", bufs=2, space="PSUM") as scratch, \
                 tc.tile_pool(name="oppsum", bufs=1, space="PSUM") as oppsum:
                for qc in range(NQC):
                    q0 = qc * QC
                    for hp in range(2):
                        h0, h1 = 2 * hp, 2 * hp + 1
                        qT = qkT[hp]
                        kT = qkT[2 + hp]
                        op = oppsum.tile([65, 2 * QC], f32, tag="op")
                        for kt in range(NKT):
                            sp = scratch.tile([P, 2 * QC], f32, tag="sp")
                            # row-tiled head pair: h0 in rows 0:64, h1 in 64:128
                            nc.tensor.matmul(
                                sp[:, 0:QC],
                                lhsT=kT[0:HD, kt * P : (kt + 1) * P],
                                rhs=qT[0:HD, q0 : q0 + QC],
                                start=True, stop=True,
                            )
                            nc.tensor.matmul(
                                sp[:, QC : 2 * QC],
                                lhsT=kT[HD:P, kt * P : (kt + 1) * P],
                                rhs=qT[HD:P, q0 : q0 + QC],
                                start=True, stop=True,
                            )
                            eb = ebp.tile([P, 2 * QC], bf16, tag="eb")
                            nc.scalar.activation(eb[:], sp[:], AF.Exp)
                            for i, h in enumerate((h0, h1)):
                                base = (h * NKT + kt) * 65
                                nc.tensor.matmul(
                                    op[:, i * QC : (i + 1) * QC],
                                    lhsT=vaug[:, base : base + 65],
                                    rhs=eb[:, i * QC : (i + 1) * QC],
                                    start=(kt == 0),
                                    stop=(kt == NKT - 1),
                                )
                        # stash numerator (bf16) + denominator row
                        for i, h in enumerate((h0, h1)):
                            ou = oup.tile([HD, QC], bf16, tag="ou")
                            nc.vector.tensor_copy(ou[:], op[0:HD, i * QC : (i + 1) * QC])
                            ou_tiles[(qc, h)] = ou
                            nc.vector.tensor_copy(
                                dden[32 * h : 32 * h + 1, q0 : q0 + QC],
                                op[HD : HD + 1, i * QC : (i + 1) * QC],
                            )
                    # normalize all 4 heads for this q-chunk
                    with nc.allow_low_precision(reason="softmax denom reciprocal"):
                        nc.vector.reciprocal(
                            drec[:, q0 : q0 + QC], dden[:, q0 : q0 + QC]
                        )
                    for h in range(NH):
                        rb = scratch.tile([P, QC], f32, tag="pp", name="rb")
                        nc.tensor.matmul(
                            rb[0:HD, :],
                            lhsT=ones_bc[32 * h : 32 * h + 1, :],
                            rhs=drec[32 * h : 32 * h + 1, q0 : q0 + QC],
                            start=True, stop=True,
                            tile_position=(32 * h, 0),
                        )
                        ou = ou_tiles.pop((qc, h))
                        with nc.allow_low_precision(reason="softmax normalize bf16"):
                            nc.vector.tensor_tensor(
                                out=obar[h][:, q0 : q0 + QC],
                                in0=ou[:],
                                in1=rb[0:HD, :],
                                op=ALU.mult,
                            )
                    # ---------------- Stage C for this q-chunk ----------------
                    for et in range(8):
                        pp = scratch.tile([P, QC], f32, tag="pp")
                        for h in range(NH):
                            nc.tensor.matmul(
                                pp[:],
                                lhsT=wp_sb[h][:, et * P : (et + 1) * P],
                                rhs=obar[h][:, q0 : q0 + QC],
                                start=(h == 0),
                                stop=(h == NH - 1),
                            )
                        ot = otp.tile([P, QC], bf16, tag="ot")
                        nc.vector.tensor_copy(ot[:], pp[:])
                        nc.sync.dma_start(
                            out=outT[et * P : (et + 1) * P, q0 : q0 + QC], in_=ot[:]
                        )

    nc.compile()
    return nc


def _get_nc():
    if "nc" not in _CACHE:
        _CACHE["nc"] = _build_program()
    return _CACHE["nc"]


def _shard_inputs(hidden_states, w_attn, w_proj):
    import ml_dtypes

    bf16 = ml_dtypes.bfloat16
    scale = 1.0 / np.sqrt(np.float32(HD))
    hidT_b = [
        np.ascontiguousarray(hidden_states[b].T).astype(bf16) for b in range(2)
    ]
    in_maps = []
    for c in range(N_CORES):
        b, g = divmod(c, 4)
        cs = slice(g * NH * HD, (g + 1) * NH * HD)
        wq = w_attn[:, 0:D][:, cs] * scale
        wk = w_attn[:, D : 2 * D][:, cs]
        wv = w_attn[:, 2 * D : 3 * D][:, cs]
        in_maps.append(
            {
                "hidT": hidT_b[b],
                "wqkv": np.ascontiguousarray(
                    np.concatenate([wq, wk, wv], axis=1)
                ).astype(bf16),
                "wp": np.ascontiguousarray(w_proj[cs, :]).astype(bf16),
            }
        )
    return in_maps


def run(hidden_states, w_attn, w_proj, trace=False):
    from concourse.bass_utils import run_bass_kernel_spmd

    nc = _get_nc()
    in_maps = _shard_inputs(hidden_states, w_attn, w_proj)
    res = run_bass_kernel_spmd(nc, in_maps, list(range(N_CORES)), trace=trace)
    parts = [res.results[c]["outT"].astype(np.float32).T for c in range(N_CORES)]
    out = np.stack(
        [
            parts[0] + parts[1] + parts[2] + parts[3],
            parts[4] + parts[5] + parts[6] + parts[7],
        ]
    ).astype(np.float32)
    return out, res


def kernel(hidden_states, w_attn, w_proj):
    out, _ = run(
        np.asarray(hidden_states), np.asarray(w_attn), np.asarray(w_proj)
    )
    return out


# revision 14
# speedup vs baseline: 1.8063x; 1.1698x over previous
"""GPT2 attention (B=2,S=2048,D=1024,H=16,hd=64, no causal mask) on 8 trn2 cores.

Sharding: core c handles batch b=c//4 and head-group g=c%4 (4 heads).
w_attn columns split per head group (Q pre-scaled by 1/sqrt(hd) on host);
w_proj rows split per head group; host sums the 4 partial c_proj outputs
per batch.

All matmul operands are bf16 (rel-err budget is 2e-2 rms; bf16 lands ~1e-3).
bf16 enables fast weight load, so per-matmul LDWEIGHTS hides behind the
matmul stream, and halves DVE/SBUF/DMA traffic vs f32.

Host-side prep: hid is shipped pre-transposed (hidT [D,S]) so the kernel
needs no PE transposes at all; the c_proj output is produced feature-major
(outT [D,S]) and transposed back on host.

Per-core dataflow:
  A) V seq-major: vps[st,:256] = hidT_tiles.T @ wv  -> vaug [128k, 65] tiles
     (col 64 pre-set to ones -> PV matmul row 64 = softmax denominator)
     Q,K feature-major: qkT[ct][128,2048] = w_slice.T @ hidT (2 heads/tile)
  B) flash loop, per (q-chunk 512, head-pair): 16 k-tiles:
     scores: two row-tiled (K=64) matmuls (head pair runs concurrently in
     the PE array) -> sp [128,1024] PSUM; one ACT exp -> eb bf16;
     PV: op[65, 512+512] += vaug.T @ eb  (row 64 = denominator)
     normalize: denominators batched -> one DVE reciprocal per q-chunk,
     ones-matmul broadcast, DVE multiply -> obar bf16
  C) c_proj feature-major: outT[et,qs] += wp_h.T @ obar_h, drain bf16,
     DMA out. Overlaps stage B of later q-chunks.
"""

import sys

import numpy as np

if "/opt/trn_rl_repo" not in sys.path:
    sys.path.insert(0, "/opt/trn_rl_repo")

S = 2048
D = 1024
P = 128
NH = 4  # heads per core
HD = 64
N_CORES = 8
QC = 512  # q-chunk width
NQC = S // QC  # 4
NKT = S // P  # 16 k-tiles

_CACHE = {}


def _build_program():
    import concourse.mybir as mybir
    from concourse import bacc
    from concourse.tile import TileContext

    bf16 = mybir.dt.bfloat16
    f32 = mybir.dt.float32
    AF = mybir.ActivationFunctionType
    ALU = mybir.AluOpType

    nc = bacc.Bacc(None, target_bir_lowering=False, debug=False)
    hidT = nc.declare_dram_parameter("hidT", [D, S], bf16, isOutput=False)
    wqkv = nc.declare_dram_parameter("wqkv", [D, 3 * NH * HD], bf16, isOutput=False)
    wp = nc.declare_dram_parameter("wp", [NH * HD, D], bf16, isOutput=False)
    outT = nc.declare_dram_parameter("outT", [D, S], bf16, isOutput=True)

    with TileContext(nc) as tc:
        with tc.tile_pool(name="const", bufs=1) as constp:
            ones_bc = constp.tile([P, HD], bf16)
            nc.gpsimd.memset(ones_bc[:], 1.0)
            # vaug: per (h, kt) a [128, 65] block: cols 0..63 = V rows,
            # col 64 = ones (PV denominator row). Pre-set everything to 1;
            # V copies overwrite cols 0..63.
            vaug = constp.tile([P, NH * NKT * 65], bf16)
            nc.gpsimd.memset(vaug[:], 1.0)

            # persistent SBUF across stages
            qkT = [constp.tile([P, S], bf16, name=f"qkT{i}") for i in range(4)]
            obar = [constp.tile([HD, S], bf16, name=f"obar{i}") for i in range(NH)]
            wp_sb = [constp.tile([HD, D], bf16, name=f"wp{i}") for i in range(NH)]
            # denominators: head h lives on partition 32h so the K=1
            # broadcast matmul's tile_position lands 32-aligned
            dden = constp.tile([97, S], f32)
            nc.gpsimd.memset(dden[:], 1.0)
            drec_f = constp.tile([97, S], f32)
            drec = constp.tile([97, S], bf16)
            for h in range(NH):
                nc.sync.dma_start(out=wp_sb[h][:], in_=wp[h * HD : (h + 1) * HD, :])

            # ---------------- Stage A ----------------
            with tc.tile_pool(name="hid_pool", bufs=1) as hidp, \
                 tc.tile_pool(name="w_pool", bufs=1) as wpool, \
                 tc.tile_pool(name="vpsum", bufs=3, space="PSUM") as vpsum, \
                 tc.tile_pool(name="qpsum", bufs=1, space="PSUM") as qpsum:
                hid_sb = [hidp.tile([P, S], bf16, name=f"hidT{i}") for i in range(8)]
                w_sb = [wpool.tile([P, 3 * NH * HD], bf16, name=f"w{i}") for i in range(8)]
                for i in range(8):
                    nc.sync.dma_start(out=hid_sb[i][:], in_=hidT[i * P : (i + 1) * P, :])
                    nc.sync.dma_start(out=w_sb[i][:], in_=wqkv[i * P : (i + 1) * P, :])

                def emit_vpass(st):
                    vps = vpsum.tile([P, NH * HD], f32, tag="vps", name="vps")
                    for dt_ in range(8):
                        nc.tensor.matmul(
                            vps[:],
                            lhsT=hid_sb[dt_][:, st * P : (st + 1) * P],
                            rhs=w_sb[dt_][:, 2 * NH * HD : 3 * NH * HD],
                            start=(dt_ == 0),
                            stop=(dt_ == 7),
                        )
                    for h in range(NH):
                        base = (h * NKT + st) * 65
                        nc.vector.tensor_copy(
                            vaug[:, base : base + HD], vps[:, h * HD : (h + 1) * HD]
                        )

                # Q,K feature-major; ct: 0=Q(h0,h1) 1=Q(h2,h3) 2=K(h0,h1) 3=K(h2,h3)
                # first flash block needs ct0/ct2 + vaug; ct1/ct3 only later
                for ct in (0, 2):
                    ps = [
                        qpsum.tile([P, QC], f32, tag=f"qk{q}", name=f"qkps{q}")
                        for q in range(4)
                    ]
                    for dt_ in range(8):
                        for q in range(4):
                            nc.tensor.matmul(
                                ps[q][:],
                                lhsT=w_sb[dt_][:, ct * P : (ct + 1) * P],
                                rhs=hid_sb[dt_][:, q * QC : (q + 1) * QC],
                                start=(dt_ == 0),
                                stop=(dt_ == 7),
                            )
                    for q in range(4):
                        nc.scalar.copy(qkT[ct][:, q * QC : (q + 1) * QC], ps[q][:])
                for st in range(NKT):
                    emit_vpass(st)
                for ct in (1, 3):
                    ps = [
                        qpsum.tile([P, QC], f32, tag=f"qk{q}", name=f"qkps{q}")
                        for q in range(4)
                    ]
                    for dt_ in range(8):
                        for q in range(4):
                            nc.tensor.matmul(
                                ps[q][:],
                                lhsT=w_sb[dt_][:, ct * P : (ct + 1) * P],
                                rhs=hid_sb[dt_][:, q * QC : (q + 1) * QC],
                                start=(dt_ == 0),
                                stop=(dt_ == 7),
                            )
                    for q in range(4):
                        nc.scalar.copy(qkT[ct][:, q * QC : (q + 1) * QC], ps[q][:])

            # ---------------- Stages B+C ----------------
            # Normalize + c_proj for q-chunk qc run as "fillers" interleaved
            # into the next chunk's flash loop (one item per kt slot) so the
            # PE stream stays dense (no HAM re-throttle) and ACT never
            # starves behind a serial proj burst.
            ou_tiles = {}
            filler_q = []

            def flush(n):
                for _ in range(min(n, len(filler_q))):
                    filler_q.pop(0)()

            with tc.tile_pool(name="ebp", bufs=3) as ebp, \
                 tc.tile_pool(name="oup", bufs=6) as oup, \
                 tc.tile_pool(name="otp", bufs=4) as otp, \
                 tc.tile_pool(name="scratch", bufs=2, space="PSUM") as scratch, \
                 tc.tile_pool(name="oppsum", bufs=1, space="PSUM") as oppsum:

                def emit_recip(qc):
                    q0 = qc * QC
                    nc.vector.reciprocal_approx_fast(
                        out=drec_f[:, q0 : q0 + QC], in_=dden[:, q0 : q0 + QC]
                    )

                def emit_cast(qc):
                    q0 = qc * QC
                    with nc.allow_low_precision(reason="softmax denom bf16"):
                        nc.vector.tensor_copy(
                            drec[:, q0 : q0 + QC], drec_f[:, q0 : q0 + QC]
                        )

                def emit_norm(qc, h):
                    q0 = qc * QC
                    rb = scratch.tile([P, QC], f32, tag="pp", name="rb")
                    nc.tensor.matmul(
                        rb[0:HD, :],
                        lhsT=ones_bc[32 * h : 32 * h + 1, :],
                        rhs=drec[32 * h : 32 * h + 1, q0 : q0 + QC],
                        start=True, stop=True,
                        tile_position=(32 * h, 0),
                    )
                    ou = ou_tiles.pop((qc, h))
                    with nc.allow_low_precision(reason="softmax normalize bf16"):
                        nc.vector.tensor_tensor(
                            out=obar[h][:, q0 : q0 + QC],
                            in0=ou[:],
                            in1=rb[0:HD, :],
                            op=ALU.mult,
                        )

                def emit_proj(qc, et):
                    q0 = qc * QC
                    pp = scratch.tile([P, QC], f32, tag="pp")
                    for h in range(NH):
                        nc.tensor.matmul(
                            pp[:],
                            lhsT=wp_sb[h][:, et * P : (et + 1) * P],
                            rhs=obar[h][:, q0 : q0 + QC],
                            start=(h == 0),
                            stop=(h == NH - 1),
                        )
                    ot = otp.tile([P, QC], bf16, tag="ot")
                    nc.vector.tensor_copy(ot[:], pp[:])
                    nc.sync.dma_start(
                        out=outT[et * P : (et + 1) * P, q0 : q0 + QC], in_=ot[:]
                    )

                import functools

                for qc in range(NQC):
                    q0 = qc * QC
                    for hp in range(2):
                        h0, h1 = 2 * hp, 2 * hp + 1
                        qT = qkT[hp]
                        kT = qkT[2 + hp]
                        op = oppsum.tile([65, 2 * QC], f32, tag="op")
                        for kt in range(NKT):
                            sp = scratch.tile([P, 2 * QC], f32, tag="sp")
                            # row-tiled head pair: h0 in rows 0:64, h1 in 64:128
                            nc.tensor.matmul(
                                sp[:, 0:QC],
                                lhsT=kT[0:HD, kt * P : (kt + 1) * P],
                                rhs=qT[0:HD, q0 : q0 + QC],
                                start=True, stop=True,
                            )
                            nc.tensor.matmul(
                                sp[:, QC : 2 * QC],
                                lhsT=kT[HD:P, kt * P : (kt + 1) * P],
                                rhs=qT[HD:P, q0 : q0 + QC],
                                start=True, stop=True,
                            )
                            eb = ebp.tile([P, 2 * QC], bf16, tag="eb")
                            nc.scalar.activation(eb[:], sp[:], AF.Exp)
                            for i, h in enumerate((h0, h1)):
                                base = (h * NKT + kt) * 65
                                nc.tensor.matmul(
                                    op[:, i * QC : (i + 1) * QC],
                                    lhsT=vaug[:, base : base + 65],
                                    rhs=eb[:, i * QC : (i + 1) * QC],
                                    start=(kt == 0),
                                    stop=(kt == NKT - 1),
                                )
                            flush(1)
                        # stash denominator row + numerator (bf16)
                        for i, h in enumerate((h0, h1)):
                            nc.vector.tensor_copy(
                                dden[32 * h : 32 * h + 1, q0 : q0 + QC],
                                op[HD : HD + 1, i * QC : (i + 1) * QC],
                            )
                            ou = oup.tile([HD, QC], bf16, tag="ou")
                            nc.vector.tensor_copy(ou[:], op[0:HD, i * QC : (i + 1) * QC])
                            ou_tiles[(qc, h)] = ou
                    filler_q.append(functools.partial(emit_recip, qc))
                    filler_q.append(functools.partial(emit_cast, qc))
                    for h in range(NH):
                        filler_q.append(functools.partial(emit_norm, qc, h))
                    for et in range(8):
                        filler_q.append(functools.partial(emit_proj, qc, et))
                flush(len(filler_q))

    nc.compile()
    return nc


def _get_nc():
    if "nc" not in _CACHE:
        _CACHE["nc"] = _build_program()
    return _CACHE["nc"]


def _shard_inputs(hidden_states, w_attn, w_proj):
    import ml_dtypes

    bf16 = ml_dtypes.bfloat16
    scale = 1.0 / np.sqrt(np.float32(HD))
    hidT_b = [
        np.ascontiguousarray(hidden_states[b].T).astype(bf16) for b in range(2)
    ]
    in_maps = []
    for c in range(N_CORES):
        b, g = divmod(c, 4)
        cs = slice(g * NH * HD, (g + 1) * NH * HD)
        wq = w_attn[:, 0:D][:, cs] * scale
        wk = w_attn[:, D : 2 * D][:, cs]
        wv = w_attn[:, 2 * D : 3 * D][:, cs]
        in_maps.append(
            {
                "hidT": hidT_b[b],
                "wqkv": np.ascontiguousarray(
                    np.concatenate([wq, wk, wv], axis=1)
                ).astype(bf16),
                "wp": np.ascontiguousarray(w_proj[cs, :]).astype(bf16),
            }
        )
    return in_maps


def run(hidden_states, w_attn, w_proj, trace=False):
    from concourse.bass_utils import run_bass_kernel_spmd

    nc = _get_nc()
    in_maps = _shard_inputs(hidden_states, w_attn, w_proj)
    res = run_bass_kernel_spmd(nc, in_maps, list(range(N_CORES)), trace=trace)
    parts = [res.results[c]["outT"].astype(np.float32).T for c in range(N_CORES)]
    out = np.stack(
        [
            parts[0] + parts[1] + parts[2] + parts[3],
            parts[4] + parts[5] + parts[6] + parts[7],
        ]
    ).astype(np.float32)
    return out, res


def kernel(hidden_states, w_attn, w_proj):
    out, _ = run(
        np.asarray(hidden_states), np.asarray(w_attn), np.asarray(w_proj)
    )
    return out


# revision 15
# speedup vs baseline: 1.8824x; 1.0421x over previous
"""GPT2 attention (B=2,S=2048,D=1024,H=16,hd=64, no causal mask) on 8 trn2 cores.

Sharding: core c handles batch b=c//4 and head-group g=c%4 (4 heads).
w_attn columns split per head group (Q pre-scaled by 1/sqrt(hd) on host);
w_proj rows split per head group; host sums the 4 partial c_proj outputs
per batch.

All matmul operands are bf16 (rel-err budget is 2e-2 rms; bf16 lands ~1e-3).
bf16 enables fast weight load, so per-matmul LDWEIGHTS hides behind the
matmul stream, and halves DVE/SBUF/DMA traffic vs f32.

Host-side prep: hid is shipped pre-transposed (hidT [D,S]) so the kernel
needs no PE transposes at all; the c_proj output is produced feature-major
(outT [D,S]) and transposed back on host.

Per-core dataflow:
  A) V seq-major: vps[st,:256] = hidT_tiles.T @ wv  -> vaug [128k, 65] tiles
     (col 64 pre-set to ones -> PV matmul row 64 = softmax denominator)
     Q,K feature-major: qkT[ct][128,2048] = w_slice.T @ hidT (2 heads/tile)
  B) flash loop, per (q-chunk 512, head-pair): 16 k-tiles:
     scores: two row-tiled (K=64) matmuls (head pair runs concurrently in
     the PE array) -> sp [128,1024] PSUM; one ACT exp -> eb bf16;
     PV: op[65, 512+512] += vaug.T @ eb  (row 64 = denominator)
     normalize: denominators batched -> one DVE reciprocal per q-chunk,
     ones-matmul broadcast, DVE multiply -> obar bf16
  C) c_proj feature-major: outT[et,qs] += wp_h.T @ obar_h, drain bf16,
     DMA out. Overlaps stage B of later q-chunks.
"""

import sys

import numpy as np

if "/opt/trn_rl_repo" not in sys.path:
    sys.path.insert(0, "/opt/trn_rl_repo")

S = 2048
D = 1024
P = 128
NH = 4  # heads per core
HD = 64
N_CORES = 8
QC = 512  # q-chunk width
NQC = S // QC  # 4
NKT = S // P  # 16 k-tiles

_CACHE = {}


def _build_program():
    import functools

    import concourse.mybir as mybir
    from concourse import bacc
    from concourse.tile import TileContext

    bf16 = mybir.dt.bfloat16
    f32 = mybir.dt.float32
    AF = mybir.ActivationFunctionType
    ALU = mybir.AluOpType

    nc = bacc.Bacc(None, target_bir_lowering=False, debug=False)
    hidT = nc.declare_dram_parameter("hidT", [D, S], bf16, isOutput=False)
    wqkv = nc.declare_dram_parameter("wqkv", [D, 3 * NH * HD], bf16, isOutput=False)
    wp = nc.declare_dram_parameter("wp", [NH * HD, D], bf16, isOutput=False)
    outT = nc.declare_dram_parameter("outT", [D, S], bf16, isOutput=True)

    with TileContext(nc) as tc:
        with tc.tile_pool(name="const", bufs=1) as constp, \
             tc.tile_pool(name="ebp", bufs=6) as ebp, \
             tc.tile_pool(name="oup", bufs=6) as oup, \
             tc.tile_pool(name="otp", bufs=4) as otp, \
             tc.tile_pool(name="scratch", bufs=2, space="PSUM") as scratch, \
             tc.tile_pool(name="oppsum", bufs=1, space="PSUM") as oppsum:
            ones_bc = constp.tile([P, HD], bf16)
            nc.gpsimd.memset(ones_bc[:], 1.0)
            # vaug: per (h, kt) a [128, 65] block: cols 0..63 = V rows,
            # col 64 = ones (PV denominator row). Pre-set everything to 1;
            # V copies overwrite cols 0..63.
            vaug = constp.tile([P, NH * NKT * 65], bf16)
            nc.gpsimd.memset(vaug[:], 1.0)

            qkT = [constp.tile([P, S], bf16, name=f"qkT{i}") for i in range(4)]
            obar = [constp.tile([HD, S], bf16, name=f"obar{i}") for i in range(NH)]
            wp_sb = [constp.tile([HD, D], bf16, name=f"wp{i}") for i in range(NH)]
            # denominators: head h lives on partition 32h so the K=1
            # broadcast matmul's tile_position lands 32-aligned
            dden = constp.tile([97, S], f32)
            nc.gpsimd.memset(dden[:], 1.0)
            drec_f = constp.tile([97, S], f32)
            drec = constp.tile([97, S], bf16)
            hid_sb = [constp.tile([P, S], bf16, name=f"hidT{i}") for i in range(8)]
            w_sb = [
                constp.tile([P, 3 * NH * HD], bf16, name=f"w{i}") for i in range(8)
            ]
            for i in range(8):
                nc.sync.dma_start(out=hid_sb[i][:], in_=hidT[i * P : (i + 1) * P, :])
                nc.gpsimd.dma_start(out=w_sb[i][:], in_=wqkv[i * P : (i + 1) * P, :])
            for h in range(NH):
                nc.gpsimd.dma_start(out=wp_sb[h][:], in_=wp[h * HD : (h + 1) * HD, :])

            # ---- emitters (stage A work routed through the shared scratch
            # pool so it can interleave with the flash loop) ----
            def emit_qk(ct, q):
                # qkT[ct][:, q*QC:...] = w[:, ct-slice].T @ hidT[:, q-slice]
                ps = scratch.tile([P, QC], f32, tag="pp", name="qk_ps")
                for dt_ in range(8):
                    nc.tensor.matmul(
                        ps[:],
                        lhsT=w_sb[dt_][:, ct * P : (ct + 1) * P],
                        rhs=hid_sb[dt_][:, q * QC : (q + 1) * QC],
                        start=(dt_ == 0),
                        stop=(dt_ == 7),
                    )
                nc.scalar.copy(qkT[ct][:, q * QC : (q + 1) * QC], ps[:])

            def emit_vpass(st):
                # V seq-major rows for k-tile st, all 4 heads + ones col
                vps = scratch.tile([P, QC], f32, tag="pp", name="vps")
                for dt_ in range(8):
                    nc.tensor.matmul(
                        vps[:, 0 : NH * HD],
                        lhsT=hid_sb[dt_][:, st * P : (st + 1) * P],
                        rhs=w_sb[dt_][:, 2 * NH * HD : 3 * NH * HD],
                        start=(dt_ == 0),
                        stop=(dt_ == 7),
                    )
                for h in range(NH):
                    base = (h * NKT + st) * 65
                    nc.vector.tensor_copy(
                        vaug[:, base : base + HD], vps[:, h * HD : (h + 1) * HD]
                    )

            def emit_recip(qc):
                q0 = qc * QC
                nc.vector.reciprocal_approx_fast(
                    out=drec_f[:, q0 : q0 + QC], in_=dden[:, q0 : q0 + QC]
                )
                with nc.allow_low_precision(reason="softmax denom bf16"):
                    nc.vector.tensor_copy(
                        drec[:, q0 : q0 + QC], drec_f[:, q0 : q0 + QC]
                    )

            def emit_norm(qc, h):
                q0 = qc * QC
                rb = scratch.tile([P, QC], f32, tag="pp", name="rb")
                nc.tensor.matmul(
                    rb[0:HD, :],
                    lhsT=ones_bc[32 * h : 32 * h + 1, :],
                    rhs=drec[32 * h : 32 * h + 1, q0 : q0 + QC],
                    start=True, stop=True,
                    tile_position=(32 * h, 0),
                )
                ou = ou_tiles.pop((qc, h))
                with nc.allow_low_precision(reason="softmax normalize bf16"):
                    nc.vector.tensor_tensor(
                        out=obar[h][:, q0 : q0 + QC],
                        in0=ou[:],
                        in1=rb[0:HD, :],
                        op=ALU.mult,
                    )

            proj_pending = {}

            def emit_proj_a(qc, et):
                q0 = qc * QC
                pp = scratch.tile([P, QC], f32, tag="pp", name="proj_pp")
                proj_pending[(qc, et)] = pp
                for h in (0, 1):
                    nc.tensor.matmul(
                        pp[:],
                        lhsT=wp_sb[h][:, et * P : (et + 1) * P],
                        rhs=obar[h][:, q0 : q0 + QC],
                        start=(h == 0),
                        stop=False,
                    )

            def emit_proj_b(qc, et):
                q0 = qc * QC
                pp = proj_pending.pop((qc, et))
                for h in (2, 3):
                    nc.tensor.matmul(
                        pp[:],
                        lhsT=wp_sb[h][:, et * P : (et + 1) * P],
                        rhs=obar[h][:, q0 : q0 + QC],
                        start=False,
                        stop=(h == 3),
                    )
                ot = otp.tile([P, QC], bf16, tag="ot")
                nc.vector.tensor_copy(ot[:], pp[:])
                nc.sync.dma_start(
                    out=outT[et * P : (et + 1) * P, q0 : q0 + QC], in_=ot[:]
                )

            ou_tiles = {}
            filler_q = []

            def flush(n):
                for _ in range(min(n, len(filler_q))):
                    filler_q.pop(0)()

            # ---------------- program ----------------
            # Q,K for head pair 0 (ct 0=Q(h0,h1), 2=K(h0,h1)), first V tiles
            for ct in (0, 2):
                for q in range(4):
                    emit_qk(ct, q)
            emit_vpass(0)
            emit_vpass(1)
            # remaining V tiles interleave into the first flash block
            filler_q.extend(
                functools.partial(emit_vpass, st) for st in range(2, NKT)
            )

            for qc in range(NQC):
                q0 = qc * QC
                for hp in range(2):
                    if qc == 0 and hp == 1:
                        # Q,K for head pair 1, needed from here on
                        for ct in (1, 3):
                            for q in range(4):
                                emit_qk(ct, q)
                    h0, h1 = 2 * hp, 2 * hp + 1
                    qT = qkT[hp]
                    kT = qkT[2 + hp]
                    op = oppsum.tile([65, 2 * QC], f32, tag="op")
                    for kt in range(NKT):
                        flush(1)
                        sp = scratch.tile([P, 2 * QC], f32, tag="sp")
                        # row-tiled head pair: h0 in rows 0:64, h1 in 64:128
                        nc.tensor.matmul(
                            sp[:, 0:QC],
                            lhsT=kT[0:HD, kt * P : (kt + 1) * P],
                            rhs=qT[0:HD, q0 : q0 + QC],
                            start=True, stop=True,
                        )
                        nc.tensor.matmul(
                            sp[:, QC : 2 * QC],
                            lhsT=kT[HD:P, kt * P : (kt + 1) * P],
                            rhs=qT[HD:P, q0 : q0 + QC],
                            start=True, stop=True,
                        )
                        eb = ebp.tile([P, 2 * QC], bf16, tag="eb")
                        nc.scalar.activation(eb[:], sp[:], AF.Exp)
                        for i, h in enumerate((h0, h1)):
                            base = (h * NKT + kt) * 65
                            nc.tensor.matmul(
                                op[:, i * QC : (i + 1) * QC],
                                lhsT=vaug[:, base : base + 65],
                                rhs=eb[:, i * QC : (i + 1) * QC],
                                start=(kt == 0),
                                stop=(kt == NKT - 1),
                            )
                    # stash denominator row + numerator (bf16)
                    for i, h in enumerate((h0, h1)):
                        nc.vector.tensor_copy(
                            dden[32 * h : 32 * h + 1, q0 : q0 + QC],
                            op[HD : HD + 1, i * QC : (i + 1) * QC],
                        )
                        ou = oup.tile([HD, QC], bf16, tag="ou")
                        nc.vector.tensor_copy(ou[:], op[0:HD, i * QC : (i + 1) * QC])
                        ou_tiles[(qc, h)] = ou
                filler_q.append(functools.partial(emit_recip, qc))
                for h in range(NH):
                    filler_q.append(functools.partial(emit_norm, qc, h))
                for et in range(8):
                    filler_q.append(functools.partial(emit_proj_a, qc, et))
                    filler_q.append(functools.partial(emit_proj_b, qc, et))
            flush(len(filler_q))

    nc.compile()
    return nc


def _get_nc():
    if "nc" not in _CACHE:
        _CACHE["nc"] = _build_program()
    return _CACHE["nc"]


def _shard_inputs(hidden_states, w_attn, w_proj):
    import ml_dtypes

    bf16 = ml_dtypes.bfloat16
    scale = 1.0 / np.sqrt(np.float32(HD))
    hidT_b = [
        np.ascontiguousarray(hidden_states[b].T).astype(bf16) for b in range(2)
    ]
    in_maps = []
    for c in range(N_CORES):
        b, g = divmod(c, 4)
        cs = slice(g * NH * HD, (g + 1) * NH * HD)
        wq = w_attn[:, 0:D][:, cs] * scale
        wk = w_attn[:, D : 2 * D][:, cs]
        wv = w_attn[:, 2 * D : 3 * D][:, cs]
        in_maps.append(
            {
                "hidT": hidT_b[b],
                "wqkv": np.ascontiguousarray(
                    np.concatenate([wq, wk, wv], axis=1)
                ).astype(bf16),
                "wp": np.ascontiguousarray(w_proj[cs, :]).astype(bf16),
            }
        )
    return in_maps


def run(hidden_states, w_attn, w_proj, trace=False):
    from concourse.bass_utils import run_bass_kernel_spmd

    nc = _get_nc()
    in_maps = _shard_inputs(hidden_states, w_attn, w_proj)
    res = run_bass_kernel_spmd(nc, in_maps, list(range(N_CORES)), trace=trace)
    parts = [res.results[c]["outT"].astype(np.float32).T for c in range(N_CORES)]
    out = np.stack(
        [
            parts[0] + parts[1] + parts[2] + parts[3],
            parts[4] + parts[5] + parts[6] + parts[7],
        ]
    ).astype(np.float32)
    return out, res


def kernel(hidden_states, w_attn, w_proj):
    out, _ = run(
        np.asarray(hidden_states), np.asarray(w_attn), np.asarray(w_proj)
    )
    return out


# revision 16
# speedup vs baseline: 1.9378x; 1.0294x over previous
"""GPT2 attention (B=2,S=2048,D=1024,H=16,hd=64, no causal mask) on 8 trn2 cores.

Sharding: core c handles batch b=c//4 and head-group g=c%4 (4 heads).
w_attn columns split per head group (Q pre-scaled by 1/sqrt(hd) on host);
w_proj rows split per head group; host sums the 4 partial c_proj outputs
per batch.

All matmul operands are bf16 (rel-err budget is 2e-2 rms; bf16 lands ~1e-3).
bf16 enables fast weight load, so per-matmul LDWEIGHTS hides behind the
matmul stream, and halves DVE/SBUF/DMA traffic vs f32.

Host-side prep: hid is shipped pre-transposed (hidT [D,S]) so the kernel
needs no PE transposes at all; the c_proj output is produced feature-major
(outT [D,S]) and transposed back on host.

Per-core dataflow:
  A) V seq-major: vps[st,:256] = hidT_tiles.T @ wv  -> vaug [128k, 65] tiles
     (col 64 pre-set to ones -> PV matmul row 64 = softmax denominator)
     Q,K feature-major: qkT[ct][128,2048] = w_slice.T @ hidT (2 heads/tile)
  B) flash loop, per (q-chunk 512, head-pair): 16 k-tiles:
     scores: two row-tiled (K=64) matmuls (head pair runs concurrently in
     the PE array) -> sp [128,1024] PSUM; one ACT exp -> eb bf16;
     PV: op[65, 512+512] += vaug.T @ eb  (row 64 = denominator)
     normalize: denominators batched -> one DVE reciprocal per q-chunk,
     ones-matmul broadcast, DVE multiply -> obar bf16
  C) c_proj feature-major: outT[et,qs] += wp_h.T @ obar_h, drain bf16,
     DMA out. Overlaps stage B of later q-chunks.
"""

import sys

import numpy as np

if "/opt/trn_rl_repo" not in sys.path:
    sys.path.insert(0, "/opt/trn_rl_repo")

S = 2048
D = 1024
P = 128
NH = 4  # heads per core
HD = 64
N_CORES = 8
QC = 512  # q-chunk width
NQC = S // QC  # 4
NKT = S // P  # 16 k-tiles

_CACHE = {}


def _build_program():
    import functools

    import concourse.mybir as mybir
    from concourse import bacc
    from concourse.tile import TileContext

    bf16 = mybir.dt.bfloat16
    f32 = mybir.dt.float32
    AF = mybir.ActivationFunctionType
    ALU = mybir.AluOpType

    nc = bacc.Bacc(None, target_bir_lowering=False, debug=False)
    hidT = nc.declare_dram_parameter("hidT", [D, S], bf16, isOutput=False)
    wqkv = nc.declare_dram_parameter("wqkv", [D, 3 * NH * HD], bf16, isOutput=False)
    wp = nc.declare_dram_parameter("wp", [NH * HD, D], bf16, isOutput=False)
    outT = nc.declare_dram_parameter("outT", [D, S], bf16, isOutput=True)

    with TileContext(nc) as tc:
        with tc.tile_pool(name="const", bufs=1) as constp, \
             tc.tile_pool(name="ebp", bufs=6) as ebp, \
             tc.tile_pool(name="oup", bufs=6) as oup, \
             tc.tile_pool(name="otp", bufs=4) as otp, \
             tc.tile_pool(name="scratch", bufs=2, space="PSUM") as scratch, \
             tc.tile_pool(name="oppsum", bufs=1, space="PSUM") as oppsum:
            ones_bc = constp.tile([P, HD], bf16)
            # vaug: per (h, kt) a [128, 65] block: cols 0..63 = V rows,
            # col 64 = ones (PV denominator row). Pre-set everything to 1;
            # V copies overwrite cols 0..63.
            vaug = constp.tile([P, NH * NKT * 65], bf16)
            qkT = [constp.tile([P, S], bf16, name=f"qkT{i}") for i in range(4)]
            # obar2/wp2: head pairs stacked on partitions (h even: 0:64,
            # h odd: 64:128) so c_proj contracts both heads in one K=128 matmul
            obar2 = [constp.tile([P, S], bf16, name=f"obar2_{i}") for i in range(2)]
            wp2_sb = [constp.tile([P, D], bf16, name=f"wp2_{i}") for i in range(2)]
            # denominators: head h lives on partition 32h so the K=1
            # broadcast matmul's tile_position lands 32-aligned
            dden = constp.tile([97, S], f32)
            drec_f = constp.tile([97, S], f32)
            drec = constp.tile([97, S], bf16)
            hid_sb = [constp.tile([P, S], bf16, name=f"hidT{i}") for i in range(8)]
            w_sb = [
                constp.tile([P, 3 * NH * HD], bf16, name=f"w{i}") for i in range(8)
            ]
            for i in range(8):
                nc.sync.dma_start(out=hid_sb[i][:], in_=hidT[i * P : (i + 1) * P, :])
                nc.gpsimd.dma_start(out=w_sb[i][:], in_=wqkv[i * P : (i + 1) * P, :])
            for hp in range(2):
                for j in range(2):
                    h = 2 * hp + j
                    nc.gpsimd.dma_start(
                        out=wp2_sb[hp][j * HD : (j + 1) * HD, :],
                        in_=wp[h * HD : (h + 1) * HD, :],
                    )
            nc.gpsimd.memset(ones_bc[:], 1.0)
            nc.gpsimd.memset(vaug[:], 1.0)
            nc.gpsimd.memset(dden[:], 1.0)

            # ---- emitters (stage A work routed through the shared scratch
            # pool so it can interleave with the flash loop) ----
            def emit_qk(ct, q):
                # qkT[ct][:, q*QC:...] = w[:, ct-slice].T @ hidT[:, q-slice]
                ps = scratch.tile([P, QC], f32, tag="pp", name="qk_ps")
                for dt_ in range(8):
                    nc.tensor.matmul(
                        ps[:],
                        lhsT=w_sb[dt_][:, ct * P : (ct + 1) * P],
                        rhs=hid_sb[dt_][:, q * QC : (q + 1) * QC],
                        start=(dt_ == 0),
                        stop=(dt_ == 7),
                    )
                nc.scalar.copy(qkT[ct][:, q * QC : (q + 1) * QC], ps[:])

            def emit_vpass(st):
                # V seq-major rows for k-tile st, all 4 heads + ones col
                vps = scratch.tile([P, QC], f32, tag="pp", name="vps")
                for dt_ in range(8):
                    nc.tensor.matmul(
                        vps[:, 0 : NH * HD],
                        lhsT=hid_sb[dt_][:, st * P : (st + 1) * P],
                        rhs=w_sb[dt_][:, 2 * NH * HD : 3 * NH * HD],
                        start=(dt_ == 0),
                        stop=(dt_ == 7),
                    )
                for h in range(NH):
                    base = (h * NKT + st) * 65
                    nc.vector.tensor_copy(
                        vaug[:, base : base + HD], vps[:, h * HD : (h + 1) * HD]
                    )

            def emit_recip(qc):
                q0 = qc * QC
                nc.vector.reciprocal_approx_fast(
                    out=drec_f[:, q0 : q0 + QC], in_=dden[:, q0 : q0 + QC]
                )
                with nc.allow_low_precision(reason="softmax denom bf16"):
                    nc.vector.tensor_copy(
                        drec[:, q0 : q0 + QC], drec_f[:, q0 : q0 + QC]
                    )

            def emit_norm(qc, h):
                q0 = qc * QC
                hp, odd = divmod(h, 2)
                r0 = odd * HD
                rb = scratch.tile([P, QC], f32, tag="pp", name="rb")
                nc.tensor.matmul(
                    rb[r0 : r0 + HD, :],
                    lhsT=ones_bc[32 * h : 32 * h + 1, :],
                    rhs=drec[32 * h : 32 * h + 1, q0 : q0 + QC],
                    start=True, stop=True,
                    tile_position=(32 * h, r0),
                )
                ou = ou_tiles.pop((qc, h))
                with nc.allow_low_precision(reason="softmax normalize bf16"):
                    nc.vector.tensor_tensor(
                        out=obar2[hp][r0 : r0 + HD, q0 : q0 + QC],
                        in0=ou[r0 : r0 + HD, :],
                        in1=rb[r0 : r0 + HD, :],
                        op=ALU.mult,
                    )

            proj_pending = {}

            def emit_proj_a(qc, et):
                q0 = qc * QC
                pp = scratch.tile([P, QC], f32, tag="pp", name="proj_pp")
                proj_pending[(qc, et)] = pp
                nc.tensor.matmul(
                    pp[:],
                    lhsT=wp2_sb[0][:, et * P : (et + 1) * P],
                    rhs=obar2[0][:, q0 : q0 + QC],
                    start=True, stop=False,
                )

            def emit_proj_b(qc, et):
                q0 = qc * QC
                pp = proj_pending.pop((qc, et))
                nc.tensor.matmul(
                    pp[:],
                    lhsT=wp2_sb[1][:, et * P : (et + 1) * P],
                    rhs=obar2[1][:, q0 : q0 + QC],
                    start=False, stop=True,
                )
                ot = otp.tile([P, QC], bf16, tag="ot")
                nc.vector.tensor_copy(ot[:], pp[:])
                nc.sync.dma_start(
                    out=outT[et * P : (et + 1) * P, q0 : q0 + QC], in_=ot[:]
                )

            ou_tiles = {}
            filler_q = []

            def flush(n):
                for _ in range(min(n, len(filler_q))):
                    filler_q.pop(0)()

            # ---------------- program ----------------
            # Q,K for head pair 0 (ct 0=Q(h0,h1), 2=K(h0,h1)), first V tiles
            for ct in (0, 2):
                for q in range(4):
                    emit_qk(ct, q)
            emit_vpass(0)
            emit_vpass(1)
            # remaining V tiles interleave into the first flash block
            filler_q.extend(
                functools.partial(emit_vpass, st) for st in range(2, NKT)
            )

            for qc in range(NQC):
                q0 = qc * QC
                for hp in range(2):
                    if qc == 0 and hp == 1:
                        # Q,K for head pair 1, needed from here on
                        for ct in (1, 3):
                            for q in range(4):
                                emit_qk(ct, q)
                    h0, h1 = 2 * hp, 2 * hp + 1
                    qT = qkT[hp]
                    kT = qkT[2 + hp]
                    op = oppsum.tile([65, 2 * QC], f32, tag="op")
                    for kt in range(NKT):
                        sp = scratch.tile([P, 2 * QC], f32, tag="sp")
                        # row-tiled head pair: h0 in rows 0:64, h1 in 64:128
                        nc.tensor.matmul(
                            sp[:, 0:QC],
                            lhsT=kT[0:HD, kt * P : (kt + 1) * P],
                            rhs=qT[0:HD, q0 : q0 + QC],
                            start=True, stop=True,
                        )
                        nc.tensor.matmul(
                            sp[:, QC : 2 * QC],
                            lhsT=kT[HD:P, kt * P : (kt + 1) * P],
                            rhs=qT[HD:P, q0 : q0 + QC],
                            start=True, stop=True,
                        )
                        eb = ebp.tile([P, 2 * QC], bf16, tag="eb")
                        nc.scalar.activation(eb[:], sp[:], AF.Exp)
                        flush(1)
                        for i, h in enumerate((h0, h1)):
                            base = (h * NKT + kt) * 65
                            nc.tensor.matmul(
                                op[:, i * QC : (i + 1) * QC],
                                lhsT=vaug[:, base : base + 65],
                                rhs=eb[:, i * QC : (i + 1) * QC],
                                start=(kt == 0),
                                stop=(kt == NKT - 1),
                            )
                    # stash denominator row + numerator (bf16)
                    for i, h in enumerate((h0, h1)):
                        nc.vector.tensor_copy(
                            dden[32 * h : 32 * h + 1, q0 : q0 + QC],
                            op[HD : HD + 1, i * QC : (i + 1) * QC],
                        )
                        r0 = (h % 2) * HD
                        ou = oup.tile([P, QC], bf16, tag="ou")
                        nc.vector.tensor_copy(
                            ou[r0 : r0 + HD, :], op[0:HD, i * QC : (i + 1) * QC]
                        )
                        ou_tiles[(qc, h)] = ou
                filler_q.append(functools.partial(emit_recip, qc))
                for h in range(NH):
                    filler_q.append(functools.partial(emit_norm, qc, h))
                for et in range(8):
                    filler_q.append(functools.partial(emit_proj_a, qc, et))
                    filler_q.append(functools.partial(emit_proj_b, qc, et))
            flush(len(filler_q))

    nc.compile()
    return nc


def _get_nc():
    if "nc" not in _CACHE:
        _CACHE["nc"] = _build_program()
    return _CACHE["nc"]


def _shard_inputs(hidden_states, w_attn, w_proj):
    import ml_dtypes

    bf16 = ml_dtypes.bfloat16
    scale = 1.0 / np.sqrt(np.float32(HD))
    hidT_b = [
        np.ascontiguousarray(hidden_states[b].T).astype(bf16) for b in range(2)
    ]
    in_maps = []
    for c in range(N_CORES):
        b, g = divmod(c, 4)
        cs = slice(g * NH * HD, (g + 1) * NH * HD)
        wq = w_attn[:, 0:D][:, cs] * scale
        wk = w_attn[:, D : 2 * D][:, cs]
        wv = w_attn[:, 2 * D : 3 * D][:, cs]
        in_maps.append(
            {
                "hidT": hidT_b[b],
                "wqkv": np.ascontiguousarray(
                    np.concatenate([wq, wk, wv], axis=1)
                ).astype(bf16),
                "wp": np.ascontiguousarray(w_proj[cs, :]).astype(bf16),
            }
        )
    return in_maps


def run(hidden_states, w_attn, w_proj, trace=False):
    from concourse.bass_utils import run_bass_kernel_spmd

    nc = _get_nc()
    in_maps = _shard_inputs(hidden_states, w_attn, w_proj)
    res = run_bass_kernel_spmd(nc, in_maps, list(range(N_CORES)), trace=trace)
    parts = [res.results[c]["outT"].astype(np.float32).T for c in range(N_CORES)]
    out = np.stack(
        [
            parts[0] + parts[1] + parts[2] + parts[3],
            parts[4] + parts[5] + parts[6] + parts[7],
        ]
    ).astype(np.float32)
    return out, res


def kernel(hidden_states, w_attn, w_proj):
    out, _ = run(
        np.asarray(hidden_states), np.asarray(w_attn), np.asarray(w_proj)
    )
    return out
